# revision 6
# baseline (speedup 1.0000x reference)
"""Bass/Trainium2 kernel for nn_LogRatio loss, data-parallel over anchor rows on 8 cores.

Math: loss = sum_{m,j,k,l} pos[j,k] * N_m[j,l] * (A[j,k] - A[j,l] - c_m)^2
with A = log(X @ X.T + eps). Expanding the square reduces everything to
per-anchor reductions S1/S2 (pos-masked) and T1S/T1C/T2S (neg-masked),
and every mask depends on labels only through the anchor's class t_j
(24 classes). So each masked row-reduction is a matmul of A (or A^2)
against a per-class 0/1 table W[l, c]: G[j, c] = sum_l A[l, j] W[l, c],
followed by a per-row pick of column c = t_j.

Device work per core (256 anchor rows): one packed bf16 DMA in
([128, 3968]: X^T rolled so this core's anchors are columns 0:256,
then 16 chunks x 120 cols of label tables), 16 sim matmuls (bf16), 8
bank-wide Ln activations -> bf16 A, 8 bf16 squares, 64 bf16 G-matmuls
accumulating into one PSUM tile [128, 240], one copy + DMA out. The
O(n) epilogue (class-column selection, diagonal correction, combine,
final sum) runs on the host from the returned G tables.
"""

import numpy as np
import ml_dtypes

N, D, KK, C = 2048, 128, 4, 24
NCORES = 8
JPC = N // NCORES   # 256 anchor rows per core
NBLK = JPC // 128   # 2 blocks of 128 rows
NCH = N // 128      # 16 l-chunks
NPAIR = NCH // 2    # 8 pairs -> [128, 512] activations
WCOL = 120          # [Wpos 24 | Wsum 24 | Wc 24 | Wpos 24 | Wsum 24]
PACKW = N           # wt starts at col 2048 in the pack
PACK_COLS = N + NCH * WCOL  # 3968
EPS = 1e-6
OMEGA = 0.1

_cache: dict = {}


def _build(repeats: int):
    import concourse.bacc as bacc
    import concourse.mybir as mybir
    import concourse.tile as tile

    f32 = mybir.dt.float32
    bf16 = mybir.dt.bfloat16
    AF = mybir.ActivationFunctionType

    nc = bacc.Bacc("TRN2", target_bir_lowering=False, debug=False)
    pack_d = nc.dram_tensor("pack", [128, PACK_COLS], bf16, kind="ExternalInput")
    gout_d = nc.dram_tensor("gout", [128, NBLK * WCOL], f32, kind="ExternalOutput")

    with tile.TileContext(nc) as tc:
        with (
            tc.tile_pool(name="const", bufs=1) as const,
            tc.tile_pool(name="inp", bufs=2) as inp,
            tc.tile_pool(name="work", bufs=3) as work,
            tc.tile_pool(name="psg", bufs=2, space="PSUM") as psg,
            tc.tile_pool(name="psim", bufs=2, space="PSUM") as psim,
        ):
            epsb = const.tile([128, 1], f32, tag="epsb")
            nc.vector.memset(epsb[:], EPS)

            def body():
                pack = inp.tile([128, PACK_COLS], bf16, tag="pack")
                nc.sync.dma_start(pack[:], pack_d[:])
                xj = pack[:, 0:JPC]
                # Full 2KB bank: PSUM accumulation groups are per 2KB "zero
                # region", and a start=True matmul lazily zeroes the whole
                # region. All four G chains share ONE group: only the first
                # matmul emitted carries start=True (zeroing the bank), only
                # the last carries stop=True; everything else accumulates.
                gbank = psg.tile([128, 512], f32, tag="g")
                g = gbank[:, 0:NBLK * WCOL]

                def sim_mms(p, sim):
                    for h in range(2):
                        c = 2 * p + h
                        nc.tensor.matmul(
                            sim[:, h * JPC:(h + 1) * JPC],
                            pack[:, c * 128:(c + 1) * 128], xj,
                            start=True, stop=True,
                        )

                sims = [None] * NPAIR
                sims[0] = psim.tile([128, 2 * JPC], f32, tag="sim", name="sim0")
                sim_mms(0, sims[0])
                for p in range(NPAIR):
                    a = work.tile([128, 2 * JPC], bf16, tag="a")
                    nc.scalar.activation(a[:], sims[p][:], AF.Ln, bias=epsb[:])
                    a2 = work.tile([128, 2 * JPC], bf16, tag="a2")
                    nc.vector.tensor_mul(a2[:], a[:], a[:])
                    if p + 1 < NPAIR:
                        sims[p + 1] = psim.tile([128, 2 * JPC], f32, tag="sim",
                                                name=f"sim{p + 1}")
                        sim_mms(p + 1, sims[p + 1])
                    for h in range(2):
                        c = 2 * p + h
                        wc = pack[:, PACKW + c * WCOL:PACKW + (c + 1) * WCOL]
                        for b in range(NBLK):
                            blk = slice(h * JPC + b * 128, h * JPC + (b + 1) * 128)
                            nc.tensor.matmul(
                                g[:, b * WCOL:b * WCOL + 72],
                                a[:, blk], wc[:, 0:72],
                                start=(c == 0 and b == 0),
                                stop=False,
                            )
                            nc.tensor.matmul(
                                g[:, b * WCOL + 72:b * WCOL + WCOL],
                                a2[:, blk], wc[:, 72:WCOL],
                                start=False,
                                stop=(c == NCH - 1 and b == NBLK - 1),
                            )

                gsb = work.tile([128, NBLK * WCOL], f32, tag="gsb")
                nc.scalar.copy(gsb[:], g[:])
                nc.sync.dma_start(gout_d[:], gsb[:])

            if repeats == 1:
                body()
            else:
                with tc.For_i(0, repeats, 1):
                    body()

    nc.compile()
    return nc


def _host_tables(labels: np.ndarray):
    """Per-class 0/1 tables [N, 120] and per-class aggregate stats."""
    lab = np.asarray(labels).astype(np.int64)
    t = lab[:, 0]
    E = (lab[:, :, None] == np.arange(C)[None, None, :]).astype(np.float32)  # [N,4,C]
    Wpos = E[:, 0]
    W0 = 1.0 - E[:, 3]
    W1 = E[:, 3] * (1.0 - E[:, 2])
    W2 = E[:, 2] * (1.0 - E[:, 1])
    W3 = E[:, 1] * (1.0 - E[:, 0])
    cm = np.array(
        [0.1 * (np.log(OMEGA + EPS) - np.log(OMEGA ** (KK - m + 1) + EPS)) for m in range(KK)],
        dtype=np.float64,
    )
    Wsum = W0 + W1 + W2 + W3
    Wc = (cm[0] * W0 + cm[1] * W1 + cm[2] * W2 + cm[3] * W3).astype(np.float32)
    Wtbl = np.concatenate([Wpos, Wsum, Wc, Wpos, Wsum], axis=1)  # [N, 120]

    colsum = np.stack([W.sum(axis=0) for W in (W0, W1, W2, W3)]).astype(np.float64)
    cnt0 = Wpos.sum(axis=0).astype(np.float64)
    NnS_c = colsum.sum(axis=0)
    NnC_c = (cm[:, None] * colsum).sum(axis=0)
    NnC2_c = ((cm ** 2)[:, None] * colsum).sum(axis=0)
    return t, Wtbl, cnt0, NnS_c, NnC_c, NnC2_c, cm


def _prep_inputs(inputs: np.ndarray, labels: np.ndarray):
    X = np.asarray(inputs, dtype=np.float32)
    t, Wtbl, *_ = _host_tables(labels)
    XTb = np.ascontiguousarray(X.T).astype(ml_dtypes.bfloat16)  # [128, 2048]
    Wb = Wtbl.astype(ml_dtypes.bfloat16)                         # [2048, 120]

    in_maps = []
    for core in range(NCORES):
        j0 = core * JPC
        xt_r = np.roll(XTb, -j0, axis=1)                  # anchors at cols 0:256
        w_r = np.roll(Wb, -j0, axis=0)                    # same l-order as xt_r
        wp = w_r.reshape(NCH, 128, WCOL).transpose(1, 0, 2).reshape(128, NCH * WCOL)
        pack = np.empty((128, PACK_COLS), dtype=ml_dtypes.bfloat16)
        pack[:, :N] = xt_r
        pack[:, N:] = wp
        in_maps.append({"pack": pack})
    return in_maps


def _host_epilogue(inputs, labels, gouts):
    """Combine per-core G tables [128, 240] into the scalar loss."""
    X = np.asarray(inputs, dtype=np.float32)
    t, _, cnt0, NnS_c, NnC_c, NnC2_c, _ = _host_tables(labels)

    # G[j, stat]: device row p, block b -> local anchor j = b*128 + p
    G = np.stack(gouts).reshape(NCORES, 128, NBLK, 5, C)     # [core, p, b, stat, c]
    G = G.transpose(0, 2, 1, 3, 4).reshape(N, 5, C)           # anchor-major
    sel = G[np.arange(N), :, t].astype(np.float64)            # [N, 5]
    S1g, T1S, T1C, S2g, T2S = sel.T

    # diagonal A[j,j] as the device computed it: bf16 X -> f32 dot -> ln -> bf16
    Xb = X.astype(ml_dtypes.bfloat16).astype(np.float32)
    simjj = (Xb * Xb).sum(axis=1)
    ajj = np.log(simjj + EPS).astype(ml_dtypes.bfloat16)
    dA = ajj.astype(np.float64)
    dA2 = (ajj * ajj).astype(ml_dtypes.bfloat16).astype(np.float64)

    S1 = S1g - dA
    S2 = S2g - dA2
    Pn = cnt0[t] - 1.0
    NnS, NnC, NnC2 = NnS_c[t], NnC_c[t], NnC2_c[t]
    L = NnS * S2 - 2.0 * S1 * (NnC + T1S) + Pn * (NnC2 + 2.0 * T1C + T2S)
    return np.float32(L.sum())


def _get_nc(repeats: int = 1):
    key = ("nc", repeats)
    if key not in _cache:
        _cache[key] = _build(repeats)
    return _cache[key]


def run_on_device(inputs, labels, repeats: int = 1):
    from concourse.bass_utils import run_bass_kernel_spmd

    nc = _get_nc(repeats)
    in_maps = _prep_inputs(inputs, labels)
    res = run_bass_kernel_spmd(nc, in_maps, list(range(NCORES)))
    gouts = [res.results[i]["gout"] for i in range(NCORES)]
    return _host_epilogue(inputs, labels, gouts)


def kernel(inputs, labels):
    total = run_on_device(inputs, labels, repeats=1)
    return (total, 0, 0, 0)


# revision 24
# speedup vs baseline: 3.6109x; 3.6109x over previous
"""Bass/Trainium2 kernel for nn_LogRatio loss, data-parallel over anchor rows on 8 cores.

Math: loss = sum_{m,j,k,l} pos[j,k] * N_m[j,l] * (A[j,k] - A[j,l] - c_m)^2
with A = log(X @ X.T + eps). Expanding the square reduces everything to
per-anchor reductions S1/S2 (pos-masked) and T1S/T1C/T2S (neg-masked),
and every mask depends on labels only through the anchor's class t_j
(24 classes). So each masked row-reduction is a matmul of A (or A^2)
against a per-class 0/1 table W[l, c]: G[j, c] = sum_l A[l, j] W[l, c],
followed by a per-row pick of column c = t_j.

Device work per core (256 anchor rows): packed bf16 input [128, 3968]
(X^T rolled so this core's anchors are columns 0:256, interleaved with
per-chunk label tables) DMA'd as four column groups so compute starts
after the first quarter lands; 16 sim matmuls (bf16), 4 quad-wide Ln
activations -> bf16 A, 4 bf16 squares, 64 bf16 G-matmuls accumulating
into one PSUM bank (single accumulation group: PSUM start=True lazily
zeroes the whole 2KB zero region, so only the first matmul starts and
only the last stops), one copy + DMA out. The O(n) epilogue (class
column selection, diagonal correction, combine, final sum) runs on the
host from the returned G tables.
"""

import numpy as np
import ml_dtypes

N, D, KK, C = 2048, 128, 4, 24
NCORES = 8
JPC = N // NCORES    # 256 anchor rows per core
NBLK = JPC // 128    # 2 blocks of 128 rows
NCH = N // 128       # 16 l-chunks
NQ = 4               # DMA/activation quads: 4 chunks each
WCOL = 72            # [Wpos 24 | Wsum 24 | Wc 24]; g2 reuses cols 0:48
GOUT = 120           # per-block G cols: [g1 72 | g2 48]
GRP = 4 * 128 + 4 * WCOL  # 800 cols per group: 4 x-chunks + their tables
PACK_COLS = NQ * GRP      # 3200
EPS = 1e-6
OMEGA = 0.1

_cache: dict = {}


def _xt_col(c):
    return (c // 4) * GRP + (c % 4) * 128


def _wt_col(c):
    return (c // 4) * GRP + 512 + (c % 4) * WCOL


def _build(repeats: int, variant: str = "v0"):
    import concourse.bacc as bacc
    import concourse.mybir as mybir
    import concourse.tile as tile

    f32 = mybir.dt.float32
    bf16 = mybir.dt.bfloat16
    AF = mybir.ActivationFunctionType

    nc = bacc.Bacc("TRN2", target_bir_lowering=False, debug=False)
    pack_d = nc.dram_tensor("pack", [128, PACK_COLS], bf16, kind="ExternalInput")
    gout_d = nc.dram_tensor("gout", [128, NBLK * GOUT], f32, kind="ExternalOutput")

    with tile.TileContext(nc) as tc:
        with (
            tc.tile_pool(name="const", bufs=1) as const,
            tc.tile_pool(name="inp", bufs=2) as inp,
            tc.tile_pool(name="work", bufs=3) as work,
            tc.tile_pool(name="psg", bufs=2, space="PSUM") as psg,
            tc.tile_pool(name="psim", bufs=2, space="PSUM") as psim,
        ):
            epsb = const.tile([128, 1], f32, tag="epsb")
            nc.vector.memset(epsb[:], EPS)

            def body():
                pack = inp.tile([128, PACK_COLS], bf16, tag="pack")
                if variant == "v4":
                    cuts = [0, 512, GRP, 2 * GRP, 3 * GRP, 4 * GRP]
                else:
                    cuts = [q * GRP for q in range(NQ + 1)]
                for lo, hi in zip(cuts[:-1], cuts[1:]):
                    nc.sync.dma_start(pack[:, lo:hi], pack_d[:, lo:hi])
                xj = pack[:, 0:JPC]
                gbank = psg.tile([128, 512], f32, tag="g")
                g = gbank[:, 0:NBLK * GOUT]

                def sim_mms(q, sim):
                    for k in range(4):
                        c = 4 * q + k
                        nc.tensor.matmul(
                            sim[:, k * JPC:(k + 1) * JPC],
                            pack[:, _xt_col(c):_xt_col(c) + 128], xj,
                            start=True, stop=True,
                        )

                sims = [None] * NQ
                sims[0] = psim.tile([128, 4 * JPC], f32, tag="sim", name="sim0")
                sim_mms(0, sims[0])
                for q in range(NQ):
                    a = work.tile([128, 4 * JPC], bf16, tag="a")
                    a2 = work.tile([128, 4 * JPC], bf16, tag="a2")
                    if variant == "v1" and q == NQ - 1:
                        for hh in range(2):
                            hs = slice(hh * 2 * JPC, (hh + 1) * 2 * JPC)
                            nc.scalar.activation(a[:, hs], sims[q][:, hs],
                                                 AF.Ln, bias=epsb[:])
                            nc.vector.tensor_mul(a2[:, hs], a[:, hs], a[:, hs])
                    else:
                        nc.scalar.activation(a[:], sims[q][:], AF.Ln, bias=epsb[:])
                        nc.vector.tensor_mul(a2[:], a[:], a[:])
                    if q + 1 < NQ:
                        sims[q + 1] = psim.tile([128, 4 * JPC], f32, tag="sim",
                                                name=f"sim{q + 1}")
                        sim_mms(q + 1, sims[q + 1])
                    for k in range(4):
                        c = 4 * q + k
                        wc = pack[:, _wt_col(c):_wt_col(c) + WCOL]
                        for b in range(NBLK):
                            blk = slice(k * JPC + b * 128, k * JPC + (b + 1) * 128)
                            nc.tensor.matmul(
                                g[:, b * GOUT:b * GOUT + 72],
                                a[:, blk], wc[:, 0:72],
                                start=(c == 0 and b == 0),
                                stop=False,
                            )
                            nc.tensor.matmul(
                                g[:, b * GOUT + 72:b * GOUT + GOUT],
                                a2[:, blk], wc[:, 0:48],
                                start=False,
                                stop=(c == NCH - 1 and b == NBLK - 1),
                            )

                gsb = work.tile([128, NBLK * GOUT], f32, tag="gsb")
                nc.scalar.copy(gsb[:], g[:])
                # Out-DMA from the Act queue (right after its copy): SP only
                # carries input-pack DMAs, so the next iteration's prefetch is
                # not head-of-line blocked behind this iteration's epilogue.
                nc.scalar.dma_start(gout_d[:], gsb[:])

            if repeats == 1:
                body()
            else:
                with tc.For_i(0, repeats, 1):
                    body()

    nc.compile()
    return nc


def _host_tables(labels: np.ndarray):
    """Per-class 0/1 tables [N, 120] and per-class aggregate stats."""
    lab = np.asarray(labels).astype(np.int64)
    t = lab[:, 0]
    E = (lab[:, :, None] == np.arange(C)[None, None, :]).astype(np.float32)  # [N,4,C]
    Wpos = E[:, 0]
    W0 = 1.0 - E[:, 3]
    W1 = E[:, 3] * (1.0 - E[:, 2])
    W2 = E[:, 2] * (1.0 - E[:, 1])
    W3 = E[:, 1] * (1.0 - E[:, 0])
    cm = np.array(
        [0.1 * (np.log(OMEGA + EPS) - np.log(OMEGA ** (KK - m + 1) + EPS)) for m in range(KK)],
        dtype=np.float64,
    )
    Wsum = W0 + W1 + W2 + W3
    Wc = (cm[0] * W0 + cm[1] * W1 + cm[2] * W2 + cm[3] * W3).astype(np.float32)
    Wtbl = np.concatenate([Wpos, Wsum, Wc], axis=1)  # [N, 72]

    colsum = np.stack([W.sum(axis=0) for W in (W0, W1, W2, W3)]).astype(np.float64)
    cnt0 = Wpos.sum(axis=0).astype(np.float64)
    NnS_c = colsum.sum(axis=0)
    NnC_c = (cm[:, None] * colsum).sum(axis=0)
    NnC2_c = ((cm ** 2)[:, None] * colsum).sum(axis=0)
    return t, Wtbl, cnt0, NnS_c, NnC_c, NnC2_c, cm


def _prep_inputs(inputs: np.ndarray, labels: np.ndarray):
    X = np.asarray(inputs, dtype=np.float32)
    t, Wtbl, *_ = _host_tables(labels)
    XTb = np.ascontiguousarray(X.T).astype(ml_dtypes.bfloat16)  # [128, 2048]
    Wb = Wtbl.astype(ml_dtypes.bfloat16)                         # [2048, 120]

    in_maps = []
    for core in range(NCORES):
        j0 = core * JPC
        xt_r = np.roll(XTb, -j0, axis=1)                  # anchors at cols 0:256
        w_r = np.roll(Wb, -j0, axis=0)                    # same l-order as xt_r
        wp = w_r.reshape(NCH, 128, WCOL).transpose(1, 0, 2)  # [128, NCH, WCOL]
        pack = np.empty((128, PACK_COLS), dtype=ml_dtypes.bfloat16)
        for q in range(NQ):
            base = q * GRP
            pack[:, base:base + 512] = xt_r[:, q * 512:(q + 1) * 512]
            pack[:, base + 512:base + GRP] = (
                wp[:, 4 * q:4 * q + 4].reshape(128, 4 * WCOL)
            )
        in_maps.append({"pack": pack})
    return in_maps


def _host_epilogue(inputs, labels, gouts):
    """Combine per-core G tables [128, 240] into the scalar loss."""
    X = np.asarray(inputs, dtype=np.float32)
    t, _, cnt0, NnS_c, NnC_c, NnC2_c, _ = _host_tables(labels)

    # G[j, stat]: device row p, block b -> local anchor j = b*128 + p
    G = np.stack(gouts).reshape(NCORES, 128, NBLK, 5, C)     # [core, p, b, stat, c]
    G = G.transpose(0, 2, 1, 3, 4).reshape(N, 5, C)           # anchor-major
    sel = G[np.arange(N), :, t].astype(np.float64)            # [N, 5]
    S1g, T1S, T1C, S2g, T2S = sel.T

    # diagonal A[j,j] as the device computed it: bf16 X -> f32 dot -> ln -> bf16
    Xb = X.astype(ml_dtypes.bfloat16).astype(np.float32)
    simjj = (Xb * Xb).sum(axis=1)
    ajj = np.log(simjj + EPS).astype(ml_dtypes.bfloat16)
    dA = ajj.astype(np.float64)
    dA2 = (ajj * ajj).astype(ml_dtypes.bfloat16).astype(np.float64)

    S1 = S1g - dA
    S2 = S2g - dA2
    Pn = cnt0[t] - 1.0
    NnS, NnC, NnC2 = NnS_c[t], NnC_c[t], NnC2_c[t]
    L = NnS * S2 - 2.0 * S1 * (NnC + T1S) + Pn * (NnC2 + 2.0 * T1C + T2S)
    return np.float32(L.sum())


def _get_nc(repeats: int = 1, variant: str = "v0"):
    key = ("nc", repeats, variant)
    if key not in _cache:
        _cache[key] = _build(repeats, variant)
    return _cache[key]


def run_on_device(inputs, labels, repeats: int = 1):
    from concourse.bass_utils import run_bass_kernel_spmd

    nc = _get_nc(repeats)
    in_maps = _prep_inputs(inputs, labels)
    res = run_bass_kernel_spmd(nc, in_maps, list(range(NCORES)))
    gouts = [res.results[i]["gout"] for i in range(NCORES)]
    return _host_epilogue(inputs, labels, gouts)


def kernel(inputs, labels):
    total = run_on_device(inputs, labels, repeats=1)
    return (total, 0, 0, 0)
